# revision 39
# baseline (speedup 1.0000x reference)
"""CodonAttention Trainium2 kernel (fp16 stream, issue-lean pipeline).

Math (per batch b, head h):
  q = x @ wq.T + bq ; k = x @ wk.T + bk ; v = x @ wv.T + bv   (head slices)
  scores = q k^T / 8 + syn_bias[codons_i, codons_j]
  out    = softmax(scores) @ v ;  final = concat_heads(out) @ wo.T + bo

Key algebraic trick: the pairwise codon bias factors through one-hots,
  pair_bias = onehot @ syn_bias @ onehot.T
so augmenting q' = [(q+bq)/8 | onehot @ syn_bias] and k' = [k | onehot] gives
  scores = q' @ k'.T        (effective head dim 128 — exactly one partition)
Softmax runs without max-subtraction (|scores| <= ~4.3, exp safe in fp32) and
the row-sum l is obtained with a ones-column in v: [O | l] = P @ [v | 1].

Sharding: 8 cores = (batch b in {0,1}) x (head h in {0..3}). Each core runs
the full attention for its (b, h), producing the UNNORMALIZED partial
projection outT = (wo_h @ O_h.T) (256, 4096) plus denominators lT (1, 4096);
the host divides, sums the 4 head partials per batch, transposes, adds bo.

Profile-driven design (trace facts from this hardware):
- Phase B is ACT-bound: exp runs 1 col/cycle @1.2GHz regardless of dtype,
  ~1.0us per [128,1024] group; the PE streams 512-row fp16 matmuls at
  ~0.42 ns/row so 4 matmuls/group (~0.87us) fit under the exp.
- Every dma_start costs ~0.6-1us of *issue* time on its queue, so DMA
  issues are spread: Sync + Scalar queues carry x/weights (hwdge),
  GpSimd carries the bias streams and all output DMAs (swdge, idle
  engine). Queue order puts chunk 0 first so compute starts ~3us in.
- Engines execute their queue in order, so late-chunk work must not sit
  in front of the attention stream: q/k projections and v transposes
  for chunks 3..7 are injected INTO the qb-0 attention stream right
  before the groups that consume them.
- The per-block output projection is deferred two groups into the next
  query block so its oacc->oall->PE chain never stalls the score
  pipeline (it runs in loose slots, PSUM bank shared with the qk
  projection pool).
- PSUM budget (8 banks): scores double-buffer 2x2 + oacc 2 + v-flip 1 +
  qkproj/outproj shared 1.
"""

import numpy as np

import concourse.mybir as mybir
import concourse.tile as tile
from concourse import bacc
from concourse.bass_utils import run_bass_kernel_spmd


def _ensure_axon_ntff_hook():
    """This image's antenv package lacks axon_hooks, so
    run_bass_kernel_spmd(trace=True) (or BASS_TRACE=1) would die on the
    import. Register a compatible module backed by the libaxon_pjrt C ABI
    so tracing works if a caller requests it."""
    import sys
    try:
        import antenv.axon_hooks  # noqa: F401
        return
    except ImportError:
        pass
    import contextlib
    import ctypes
    import types
    try:
        lib = ctypes.CDLL("/opt/axon/libaxon_pjrt.so")
        has = hasattr(lib, "axon_start_nrt_profile")
    except OSError:
        has = False
    if has:
        lib.axon_start_nrt_profile.argtypes = [ctypes.POINTER(ctypes.c_int64),
                                               ctypes.c_size_t]
        lib.axon_start_nrt_profile.restype = ctypes.c_int64
        lib.axon_stop_nrt_profile.argtypes = [ctypes.c_char_p]
        lib.axon_stop_nrt_profile.restype = ctypes.c_int64

        @contextlib.contextmanager
        def _hook(output_dir, device_ids):
            import jax
            jax.devices()
            if device_ids:
                ids = (ctypes.c_int64 * len(device_ids))(*device_ids)
                rc = lib.axon_start_nrt_profile(ids, len(device_ids))
            else:
                rc = lib.axon_start_nrt_profile(None, 0)
            if rc != 0:
                raise RuntimeError(f"axon_start_nrt_profile rc={rc}")
            try:
                yield
            finally:
                lib.axon_stop_nrt_profile(str(output_dir).encode())
    else:
        _hook = None

    mod = types.ModuleType("antenv.axon_hooks")
    _state = {"hook": _hook}
    mod.get_axon_ntff_profile_hook = lambda: _state["hook"]
    mod.set_axon_ntff_profile_hook = lambda h: _state.__setitem__("hook", h)
    sys.modules["antenv.axon_hooks"] = mod


_ensure_axon_ntff_hook()

B, S, HID, NH, D = 2, 4096, 256, 4, 64
DV = D + 16        # v cols + ones col + pad: DoubleRow requires the k-tile
                   # stride (outermost lhsT free step) to be 16B-aligned
LCOL = D           # index of the ones column inside a v tile
QB = 512           # query block (free dim of score matmuls)
KT = 128           # key tile (partition dim of transposed scores)
CH = 512           # x chunk width
NCH = S // CH      # 8
NQB = S // QB      # 8
NKT = S // KT      # 32
GRP = 2            # key tiles per exp group (2 PSUM banks per group)
NG = NKT // GRP    # 16 groups per query block

F32 = mybir.dt.float32
F32R = mybir.dt.float32r
F16 = mybir.dt.float16
F8E4 = mybir.dt.float8e4
Exp = mybir.ActivationFunctionType.Exp
DoubleRow = mybir.MatmulPerfMode.DoubleRow


def build_program():
    nc = bacc.Bacc("TRN2", target_bir_lowering=False, debug=False, num_devices=8)

    def di(name, shape, dt=F16):
        return nc.dram_tensor(name, shape, dt, kind="ExternalInput").ap()

    xT = di("xT", [HID, S])            # x[b].T
    wqk = di("wqk", [128, 256])        # [wqT_hi/8|wkT_hi ; wqT_lo/8|wkT_lo]
    wv2 = di("wv2", [128, 2 * DV])     # [wvT_hi | wvT_lo], col 64.. pad 0
    bias2 = di("bias2", [128, 2], F32) # col0 = [bq/8; bk]
    bsynT = di("bsynT", [D, S])        # (onehot @ syn_bias).T
    onehotT = di("onehotT", [D, S])
    woT = di("woT", [D, HID])          # wo[:, hslice].T
    outT = nc.dram_tensor("outT", [HID, S], F16, kind="ExternalOutput").ap()
    lT = nc.dram_tensor("lT", [1, S], F32, kind="ExternalOutput").ap()

    with tile.TileContext(nc) as tc:
        _body(tc, xT, wqk, wv2, bias2, bsynT, onehotT, woT, outT, lT)
    nc.compile()
    return nc


def _body(tc, xT, wqk, wv2, bias2, bsynT, onehotT, woT, outT, lT):
    nc = tc.nc
    mm = nc.tensor.matmul

    with (
        tc.tile_pool(name="const", bufs=1) as constp,
        tc.tile_pool(name="big", bufs=1) as bigp,
        tc.tile_pool(name="pt", bufs=12) as ptp,
        tc.tile_pool(name="ob", bufs=2) as obp,
    ):
        # ---- constants ----
        wqk_sb = constp.tile([128, 256], F16, name="wqk_sb", tag="wqk_sb")
        wv_sb = constp.tile([128, 2 * DV], F16, name="wv_sb", tag="wv_sb")
        b2_sb = constp.tile([128, 2], F32, name="b2_sb", tag="b2_sb")
        wo_sb = constp.tile([D, HID], F16, name="wo_sb", tag="wo_sb")
        scr = constp.tile([1, 1], F32, name="scr", tag="scr")

        # persistent activations (subregion deps make slices per-chunk)
        xc0 = [bigp.tile([128, CH], F16, name=f"xc0_{c}", tag=f"xc0_{c}")
               for c in range(NCH)]
        xc1 = [bigp.tile([128, CH], F16, name=f"xc1_{c}", tag=f"xc1_{c}")
               for c in range(NCH)]
        qTt = bigp.tile([128, S], F16, name="qTt", tag="qTt")  # 0:64 q/8, 64:128 bsynT
        kTt = bigp.tile([128, S], F16, name="kTt", tag="kTt")  # 0:64 k,   64:128 onehotT
        # v' key-major, fp8 (PV runs in DoubleRow perf mode: 2 key tiles
        # per matmul at 0.5 cycles/row). [partition=key, ktile, dv]
        vb = bigp.tile([128, NKT, DV], F8E4, name="vb", tag="vb")
        oall = bigp.tile([D, S], F16, name="oall", tag="oall")
        l_sb = bigp.tile([1, S], F32, name="l_sb", tag="l_sb")

        # ---- DMA issues. Rules learned from traces:
        # 1. Dependencies on DMA completions get coarsened by semaphore
        #    ring reuse, so ALL dma_starts must be emitted in global
        #    deadline order — a late-needed transfer emitted early poisons
        #    the waits of critical ones.
        # 2. The shared DMA engines are bandwidth-limited early; bulk
        #    transfers issued up-front crowd out the chunk-0 criticals.
        #    So only the critical wave is issued here; the chunk 2..7
        #    trios are issued from inside the stream (gpsimd queue, which
        #    has its own semaphore pool and is otherwise idle).
        # 3. The scalar queue only carries issues that complete before the
        #    exp stream starts (it is the ACT/bottleneck queue).
        nc.sync.dma_start(wqk_sb[:], wqk[:])
        nc.scalar.dma_start(xc1[0][:], xT[128:256, 0:CH])
        nc.sync.dma_start(xc0[0][:], xT[0:128, 0:CH])
        nc.scalar.dma_start(kTt[64:128, 0:CH], onehotT[:, 0:CH])
        nc.sync.dma_start(qTt[64:128, 0:CH], bsynT[:, 0:CH])
        nc.scalar.dma_start(b2_sb[:], bias2[:])
        nc.scalar.dma_start(wv_sb[:], wv2[:])
        nc.sync.dma_start(xc0[1][:], xT[0:128, CH:2 * CH])
        nc.sync.dma_start(xc1[1][:], xT[128:256, CH:2 * CH])
        nc.sync.dma_start(qTt[64:128, CH:2 * CH], bsynT[:, CH:2 * CH])
        nc.sync.dma_start(kTt[64:128, CH:2 * CH], onehotT[:, CH:2 * CH])
        for c in range(2, NCH):
            cs = slice(c * CH, (c + 1) * CH)
            nc.sync.dma_start(xc0[c][:], xT[0:128, cs])
            nc.sync.dma_start(xc1[c][:], xT[128:256, cs])

        def emit_bias_dma(c):
            cs = slice(c * CH, (c + 1) * CH)
            nc.gpsimd.dma_start(qTt[64:128, cs], bsynT[:, cs])
            nc.gpsimd.dma_start(kTt[64:128, cs], onehotT[:, cs])

        # warm the ACT exp table (~2.7us) while projections run
        nc.scalar.activation(scr[:], b2_sb[0:1, 0:1], Exp)

        with (
            tc.tile_pool(name="psB", bufs=2, space="PSUM") as psB,
            tc.tile_pool(name="psAcc", bufs=2, space="PSUM") as psAcc,
            tc.tile_pool(name="psX", bufs=2, space="PSUM") as psX,
        ):
            # ---- helpers ----
            def emit_qk(c, pool, on_act=False):
                cs = slice(c * CH, (c + 1) * CH)
                qkp = pool.tile([128, CH], F32, name="qkp",
                                tag="oacc" if pool is psAcc else "px")
                mm(qkp[:], wqk_sb[:, 0:128], xc0[c][:], start=True, stop=False)
                mm(qkp[:], wqk_sb[:, 128:256], xc1[c][:], start=False,
                   stop=True)
                # kT eviction FIRST: during qb0 the next score group waits
                # only on kTt (qTt chunk c isn't read until qb c), so the
                # qT eviction stays off the critical chain.
                nc.vector.tensor_scalar_add(kTt[0:D, cs], qkp[D:128, :],
                                            b2_sb[D:128, 0:1])
                nc.vector.tensor_scalar_add(qTt[0:D, cs], qkp[0:D, :],
                                            b2_sb[0:D, 0:1])

            def emit_v(c):
                # v' computed DIRECTLY key-major: out[key, d] with the x
                # chunk slice as stationary and the wv half as moving (68
                # moving rows per matmul, vs 512-row projections plus PE
                # transposes). bv is folded into bo on the HOST (it only
                # shifts the normalized output by a constant per head), so
                # only 2 matmuls per key tile; the denominator ones column
                # is planted by gpsimd memsets after the eviction.
                vtr = psX.tile([128, 4, DV], F32, name="vtr", tag="px")
                for m in range(4):
                    ks = slice(m * KT, (m + 1) * KT)
                    mm(vtr[:, m:m + 1, :], xc0[c][:, ks], wv_sb[:, 0:DV],
                       start=True, stop=False)
                    mm(vtr[:, m:m + 1, :], xc1[c][:, ks], wv_sb[:, DV:2 * DV],
                       start=False, stop=True)
                nc.vector.tensor_copy(vb[:, 4 * c:4 * c + 4, :], vtr[:])
                for m in range(4):
                    j = 4 * c + m
                    nc.gpsimd.memset(vb[:, j:j + 1, LCOL:LCOL + 1], 1.0)

            oaccs = {}

            def proj_steps(qb):
                """Deferred output projection for query block qb; the oacc
                eviction happens immediately (DVE is idle), the PE matmuls
                run later in loose slots of the next block."""
                qsl = slice(qb * QB, (qb + 1) * QB)
                oacc = oaccs.pop(qb)
                nc.vector.tensor_copy(oall[:, qsl], oacc[0:D, :])
                nc.vector.tensor_copy(l_sb[:, qsl], oacc[LCOL:LCOL + 1, :])

                def s1():
                    pj = psX.tile([128, QB], F32, name="pj", tag="px")
                    ob = obp.tile([128, QB], F16, name="ob", tag="ob")
                    mm(pj[:], wo_sb[:, 0:128], oall[:, qsl],
                       start=True, stop=True)
                    nc.vector.tensor_copy(ob[:], pj[:])
                    nc.gpsimd.dma_start(outT[0:128, qsl], ob[:])

                def s2():
                    pj = psX.tile([128, QB], F32, name="pj", tag="px")
                    ob = obp.tile([128, QB], F16, name="ob", tag="ob")
                    mm(pj[:], wo_sb[:, 128:256], oall[:, qsl],
                       start=True, stop=True)
                    nc.vector.tensor_copy(ob[:], pj[:])
                    nc.gpsimd.dma_start(outT[128:256, qsl], ob[:])
                    nc.gpsimd.dma_start(lT[:, qsl], l_sb[:, qsl])

                return [s1, s2]

            def proj_last(qb):
                """Final block: same halves; casts split across Vector and
                GpSimd so they run in parallel, and the output DMAs go on
                the two hwdge queues (sync + the now-idle scalar) so no
                slow swdge drain sits at the very end."""
                qsl = slice(qb * QB, (qb + 1) * QB)
                oacc = oaccs.pop(qb)
                nc.vector.tensor_copy(oall[:, qsl], oacc[0:D, :])
                # ACT is idle once the exp stream ends: it takes the l copy
                # and the half-1 cast so both tail chains run in parallel
                # with Vector's (oacc evict + half-0 cast).
                nc.scalar.activation(l_sb[:, qsl], oacc[LCOL:LCOL + 1, :],
                                     mybir.ActivationFunctionType.Copy)
                nc.sync.dma_start(lT[:, qsl], l_sb[:, qsl])
                for half, ofs in ((0, 0), (1, 128)):
                    pj = psX.tile([128, QB], F32, name="pjl", tag="px")
                    ob = obp.tile([128, QB], F16, name="obl", tag="ob")
                    mm(pj[:], wo_sb[:, ofs:ofs + 128], oall[:, qsl],
                       start=True, stop=True)
                    if half == 0:
                        nc.vector.tensor_copy(ob[:], pj[:])
                    else:
                        nc.scalar.activation(
                            ob[:], pj[:], mybir.ActivationFunctionType.Copy)
                    q0 = qb * QB
                    eng = (nc.sync, nc.scalar) if half == 0 else \
                          (nc.gpsimd, nc.sync)
                    eng[0].dma_start(outT[ofs:ofs + 128, q0:q0 + 256],
                                     ob[:, 0:256])
                    eng[1].dma_start(outT[ofs:ofs + 128, q0 + 256:q0 + 512],
                                     ob[:, 256:512])

            # PE p-state warmup: dummy matmuls on the first-arrived weights
            # bridge the gap until the x chunk-0 DMA lands (so qk0 doesn't
            # run at the cold 0.65GHz p-state). A few more after qk0 keep
            # the PE busy while the DVE bias-add produces qTt/kTt chunk 0,
            # preserving the p-state ramp into the score stream.
            warm = psX.tile([128, 256], F32, name="warm", tag="px")
            for _ in range(10):
                mm(warm[:], wqk_sb[:, 0:128], wqk_sb[:], start=True, stop=True)
            emit_qk(0, psX)
            for _ in range(4):
                mm(warm[:], wqk_sb[:, 0:128], wqk_sb[:], start=True, stop=True)

            # ---- injected work, placed just ahead of each deadline:
            # kTt chunk c feeds score groups 2c..2c+1 -> qk(c) at group
            # 2c-1; vb chunk c is first read by PV(2c) which drains at
            # group 2c+PVLAG -> emit_v(c) at 2c+2. PV is lagged by a deep
            # PVLAG=10 so qb0 carries no PV work at all -> the PE (which
            # also runs all the injected projections at the not-yet-ramped
            # p-state) can keep the score stream ahead of ACT.
            inject = {
                0: [lambda: emit_bias_dma(2), lambda: emit_qk(1, psAcc)],
                1: [lambda: emit_bias_dma(3), lambda: emit_qk(2, psX)],
                2: [lambda: emit_bias_dma(4), lambda: emit_v(0)],
                3: [lambda: emit_qk(3, psAcc)],
                4: [lambda: emit_bias_dma(5), lambda: emit_v(1)],
                5: [lambda: emit_qk(4, psAcc)],
                6: [lambda: emit_bias_dma(6), lambda: emit_v(2)],
                7: [lambda: emit_qk(5, psAcc)],
                8: [lambda: emit_bias_dma(7), lambda: emit_v(3)],
                9: [lambda: emit_qk(6, psX)],
                10: [lambda: nc.gpsimd.dma_start(wo_sb[:], woT[:]),
                     lambda: emit_v(4)],
                11: [lambda: emit_qk(7, psX)],
                12: [lambda: emit_v(5)],
                14: [lambda: emit_v(6)],
            }
            inject_qb1 = {
                0: [lambda: emit_v(7)],
            }

            # ---- attention stream (PV lags scores by PVLAG groups) ----
            PVLAG = 10
            pv_queue = []
            pending_proj = []

            def emit_pv(qb, gi, p3):
                if gi == 0:
                    oaccs[qb] = psAcc.tile([DV, QB], F32, name="oacc",
                                           tag="oacc")
                oacc = oaccs[qb]
                # One fp8 DoubleRow matmul covers both key tiles of the
                # group (2x128 contraction) at 0.5 cycles/row.
                j = GRP * gi
                mm(oacc[:], vb[:, j:j + GRP, :], p3[:],
                   start=(j == 0), stop=(j == NKT - GRP),
                   perf_mode=DoubleRow)

            done_qb = {}

            def drain_one_pv():
                qb0_, gi0_, p30_ = pv_queue.pop(0)
                emit_pv(qb0_, gi0_, p30_)
                if gi0_ == NG - 1:
                    done_qb[qb0_] = True

            for qb in range(NQB):
                qsl = slice(qb * QB, (qb + 1) * QB)
                for gi in range(NG):
                    if qb == 0:
                        for thunk in inject.get(gi, ()):
                            thunk()
                    elif qb == 1:
                        for thunk in inject_qb1.get(gi, ()):
                            thunk()
                    s3 = psB.tile([128, GRP, QB], F32, name="s3", tag="s3")
                    for m in range(GRP):
                        j = GRP * gi + m
                        jl = slice(j * KT, (j + 1) * KT)
                        mm(s3[:, m:m + 1, :], kTt[:, jl], qTt[:, qsl],
                           start=True, stop=True)
                    p3 = ptp.tile([128, GRP, QB], F8E4, name="p3", tag="p3")
                    # flat (depth-1) APs for the ACT engine; the 3D tile
                    # shape only matters for the DoubleRow PV operand view
                    nc.scalar.activation(p3[:].rearrange("p a b -> p (a b)"),
                                         s3[:].rearrange("p a b -> p (a b)"),
                                         Exp)
                    pv_queue.append((qb, gi, p3))
                    # Last three blocks: taper the PV lag gradually (the
                    # extra drains sit mid-block, away from the boundary
                    # where the PE p-state is still recovering) so the
                    # post-stream tail is one PV group instead of PVLAG+1.
                    if qb < NQB - 3:
                        lag = PVLAG
                    elif qb == NQB - 3:
                        lag = PVLAG - min(2, max(0, gi - 8))
                    elif qb == NQB - 2:
                        lag = 8 - min(3, max(0, gi - 5))
                    else:
                        lag = 5 - min(4, max(0, gi - 4))
                    drains = 0
                    while len(pv_queue) > lag and drains < 2:
                        drain_one_pv()
                        drains += 1
                    # previous block done accumulating? evict + start proj
                    # (extend, never replace: with the taper, done_qb can
                    # fire while the previous block's s2 is still pending)
                    if done_qb.pop(qb - 1, None):
                        pending_proj.extend(proj_steps(qb - 1))
                    if pending_proj and (gi % 6 == 5 or
                                         (qb == NQB - 1 and gi % 3 == 2)):
                        pending_proj.pop(0)()
            while pv_queue:
                drain_one_pv()
            for step in pending_proj:
                step()
            proj_last(NQB - 1)


_NC_CACHE = {}


def _get_program():
    if "nc" not in _NC_CACHE:
        _NC_CACHE["nc"] = build_program()
    return _NC_CACHE["nc"]


def make_in_maps(x, codons, syn_bias, wq, bq, wk, bk, wv, bv, wo):
    in_maps = []
    for core in range(8):
        b, h = divmod(core, NH)
        hsl = slice(h * D, (h + 1) * D)
        cod = codons[b]
        onehotT = np.zeros((D, S), np.float16)
        onehotT[cod, np.arange(S)] = 1.0
        # [wqT/8 | wkT] packed as [hi-half ; lo-half] -> [128, 256]
        wqk_full = np.concatenate([wq[hsl, :].T / 8.0, wk[hsl, :].T], axis=1)
        wqk = np.concatenate([wqk_full[0:128], wqk_full[128:256]], axis=1)
        wvp = np.concatenate(
            [wv[hsl, :].T, np.zeros((HID, DV - D), np.float32)], axis=1)
        wv2 = np.concatenate([wvp[0:128], wvp[128:256]], axis=1)
        bias2 = np.zeros((128, 2), np.float32)
        bias2[:, 0] = np.concatenate([bq[hsl] / 8.0, bk[hsl]])
        in_maps.append({
            "xT": x[b].T.astype(np.float16),
            "wqk": wqk.astype(np.float16),
            "wv2": wv2.astype(np.float16),
            "bias2": bias2,
            "bsynT": np.ascontiguousarray(syn_bias.T[:, cod]).astype(np.float16),
            "onehotT": onehotT,
            "woT": wo[:, hsl].T.astype(np.float16),
        })
    return in_maps


def kernel_run(inputs, trace=False):
    x = np.asarray(inputs["x"], np.float32)
    codons = np.asarray(inputs["codons"]).astype(np.int64)
    syn_bias = np.asarray(inputs["syn_bias"], np.float32)
    wq = np.asarray(inputs["wq"], np.float32)
    bq = np.asarray(inputs["bq"], np.float32)
    wk = np.asarray(inputs["wk"], np.float32)
    bk = np.asarray(inputs["bk"], np.float32)
    wv = np.asarray(inputs["wv"], np.float32)
    bv = np.asarray(inputs["bv"], np.float32)
    wo = np.asarray(inputs["wo"], np.float32)
    bo = np.asarray(inputs["bo"], np.float32)

    nc = _get_program()
    in_maps = make_in_maps(x, codons, syn_bias, wq, bq, wk, bk, wv, bv, wo)
    res = run_bass_kernel_spmd(nc, in_maps, core_ids=list(range(8)), trace=trace)

    # v' is computed WITHOUT bv on-device; after normalization the bias
    # contributes exactly wo @ bv per sequence position, so fold it into bo.
    bo_eff = bo + wo @ bv
    out = np.empty((B, S, HID), np.float32)
    for b in range(B):
        acc = None
        for h in range(NH):
            r = res.results[NH * b + h]
            part = r["outT"].astype(np.float32) / r["lT"]   # normalize per head
            acc = part if acc is None else acc + part
        out[b] = acc.T + bo_eff
    return out, res


def kernel(**inputs):
    out, _ = kernel_run(inputs, trace=False)
    return out



# revision 42
# speedup vs baseline: 1.0092x; 1.0092x over previous
"""CodonAttention Trainium2 kernel (fp16 stream, issue-lean pipeline).

Math (per batch b, head h):
  q = x @ wq.T + bq ; k = x @ wk.T + bk ; v = x @ wv.T + bv   (head slices)
  scores = q k^T / 8 + syn_bias[codons_i, codons_j]
  out    = softmax(scores) @ v ;  final = concat_heads(out) @ wo.T + bo

Key algebraic trick: the pairwise codon bias factors through one-hots,
  pair_bias = onehot @ syn_bias @ onehot.T
so augmenting q' = [(q+bq)/8 | onehot @ syn_bias] and k' = [k | onehot] gives
  scores = q' @ k'.T        (effective head dim 128 — exactly one partition)
Softmax runs without max-subtraction (|scores| <= ~4.3, exp safe in fp32) and
the row-sum l is obtained with a ones-column in v: [O | l] = P @ [v | 1].

Sharding: 8 cores = (batch b in {0,1}) x (head h in {0..3}). Each core runs
the full attention for its (b, h), producing the UNNORMALIZED partial
projection outT = (wo_h @ O_h.T) (256, 4096) plus denominators lT (1, 4096);
the host divides, sums the 4 head partials per batch, transposes, adds bo.

Profile-driven design (trace facts from this hardware):
- Phase B is ACT-bound: exp runs 1 col/cycle @1.2GHz regardless of dtype,
  ~1.0us per [128,1024] group; the PE streams 512-row fp16 matmuls at
  ~0.42 ns/row so 4 matmuls/group (~0.87us) fit under the exp.
- Every dma_start costs ~0.6-1us of *issue* time on its queue, so DMA
  issues are spread: Sync + Scalar queues carry x/weights (hwdge),
  GpSimd carries the bias streams and all output DMAs (swdge, idle
  engine). Queue order puts chunk 0 first so compute starts ~3us in.
- Engines execute their queue in order, so late-chunk work must not sit
  in front of the attention stream: q/k projections and v transposes
  for chunks 3..7 are injected INTO the qb-0 attention stream right
  before the groups that consume them.
- The per-block output projection is deferred two groups into the next
  query block so its oacc->oall->PE chain never stalls the score
  pipeline (it runs in loose slots, PSUM bank shared with the qk
  projection pool).
- PSUM budget (8 banks): scores double-buffer 2x2 + oacc 2 + v-flip 1 +
  qkproj/outproj shared 1.
"""

import numpy as np

import concourse.mybir as mybir
import concourse.tile as tile
from concourse import bacc
from concourse.bass_utils import run_bass_kernel_spmd


def _ensure_axon_ntff_hook():
    """This image's antenv package lacks axon_hooks, so
    run_bass_kernel_spmd(trace=True) (or BASS_TRACE=1) would die on the
    import. Register a compatible module backed by the libaxon_pjrt C ABI
    so tracing works if a caller requests it."""
    import sys
    try:
        import antenv.axon_hooks  # noqa: F401
        return
    except ImportError:
        pass
    import contextlib
    import ctypes
    import types
    try:
        lib = ctypes.CDLL("/opt/axon/libaxon_pjrt.so")
        has = hasattr(lib, "axon_start_nrt_profile")
    except OSError:
        has = False
    if has:
        lib.axon_start_nrt_profile.argtypes = [ctypes.POINTER(ctypes.c_int64),
                                               ctypes.c_size_t]
        lib.axon_start_nrt_profile.restype = ctypes.c_int64
        lib.axon_stop_nrt_profile.argtypes = [ctypes.c_char_p]
        lib.axon_stop_nrt_profile.restype = ctypes.c_int64

        @contextlib.contextmanager
        def _hook(output_dir, device_ids):
            import jax
            jax.devices()
            if device_ids:
                ids = (ctypes.c_int64 * len(device_ids))(*device_ids)
                rc = lib.axon_start_nrt_profile(ids, len(device_ids))
            else:
                rc = lib.axon_start_nrt_profile(None, 0)
            if rc != 0:
                raise RuntimeError(f"axon_start_nrt_profile rc={rc}")
            try:
                yield
            finally:
                lib.axon_stop_nrt_profile(str(output_dir).encode())
    else:
        _hook = None

    mod = types.ModuleType("antenv.axon_hooks")
    _state = {"hook": _hook}
    mod.get_axon_ntff_profile_hook = lambda: _state["hook"]
    mod.set_axon_ntff_profile_hook = lambda h: _state.__setitem__("hook", h)
    sys.modules["antenv.axon_hooks"] = mod


_ensure_axon_ntff_hook()

B, S, HID, NH, D = 2, 4096, 256, 4, 64
DV = D + 16        # v cols + ones col + pad: DoubleRow requires the k-tile
                   # stride (outermost lhsT free step) to be 16B-aligned
LCOL = D           # index of the ones column inside a v tile
QB = 512           # query block (free dim of score matmuls)
KT = 128           # key tile (partition dim of transposed scores)
CH = 512           # x chunk width
NCH = S // CH      # 8
NQB = S // QB      # 8
NKT = S // KT      # 32
GRP = 2            # key tiles per exp group (2 PSUM banks per group)
NG = NKT // GRP    # 16 groups per query block

F32 = mybir.dt.float32
F32R = mybir.dt.float32r
F16 = mybir.dt.float16
F8E4 = mybir.dt.float8e4
Exp = mybir.ActivationFunctionType.Exp
DoubleRow = mybir.MatmulPerfMode.DoubleRow


def build_program():
    nc = bacc.Bacc("TRN2", target_bir_lowering=False, debug=False, num_devices=8)

    def di(name, shape, dt=F16):
        return nc.dram_tensor(name, shape, dt, kind="ExternalInput").ap()

    xT = di("xT", [HID, S])            # x[b].T
    wqk = di("wqk", [128, 256])        # [wqT_hi/8|wkT_hi ; wqT_lo/8|wkT_lo]
    wv2 = di("wv2", [128, 2 * DV])     # [wvT_hi | wvT_lo], col 64.. pad 0
    bias2 = di("bias2", [128, 2], F32) # col0 = [bq/8; bk]
    bsynT = di("bsynT", [D, S])        # (onehot @ syn_bias).T
    onehotT = di("onehotT", [D, S])
    woT = di("woT", [D, HID])          # wo[:, hslice].T
    outT = nc.dram_tensor("outT", [HID, S], F16, kind="ExternalOutput").ap()
    lT = nc.dram_tensor("lT", [1, S], F32, kind="ExternalOutput").ap()

    with tile.TileContext(nc) as tc:
        _body(tc, xT, wqk, wv2, bias2, bsynT, onehotT, woT, outT, lT)
    nc.compile()
    return nc


def _body(tc, xT, wqk, wv2, bias2, bsynT, onehotT, woT, outT, lT):
    nc = tc.nc
    mm = nc.tensor.matmul

    with (
        tc.tile_pool(name="const", bufs=1) as constp,
        tc.tile_pool(name="big", bufs=1) as bigp,
        tc.tile_pool(name="pt", bufs=12) as ptp,
        tc.tile_pool(name="ob", bufs=2) as obp,
    ):
        # ---- constants ----
        wqk_sb = constp.tile([128, 256], F16, name="wqk_sb", tag="wqk_sb")
        wv_sb = constp.tile([128, 2 * DV], F16, name="wv_sb", tag="wv_sb")
        b2_sb = constp.tile([128, 2], F32, name="b2_sb", tag="b2_sb")
        wo_sb = constp.tile([D, HID], F16, name="wo_sb", tag="wo_sb")
        scr = constp.tile([1, 1], F32, name="scr", tag="scr")

        # persistent activations (subregion deps make slices per-chunk)
        xc0 = [bigp.tile([128, CH], F16, name=f"xc0_{c}", tag=f"xc0_{c}")
               for c in range(NCH)]
        xc1 = [bigp.tile([128, CH], F16, name=f"xc1_{c}", tag=f"xc1_{c}")
               for c in range(NCH)]
        qTt = bigp.tile([128, S], F16, name="qTt", tag="qTt")  # 0:64 q/8, 64:128 bsynT
        kTt = bigp.tile([128, S], F16, name="kTt", tag="kTt")  # 0:64 k,   64:128 onehotT
        # v' key-major, fp8 (PV runs in DoubleRow perf mode: 2 key tiles
        # per matmul at 0.5 cycles/row). [partition=key, ktile, dv]
        vb = bigp.tile([128, NKT, DV], F8E4, name="vb", tag="vb")
        oall = bigp.tile([D, S], F16, name="oall", tag="oall")
        l_sb = bigp.tile([1, S], F32, name="l_sb", tag="l_sb")

        # ---- DMA issues. Rules learned from traces:
        # 1. Dependencies on DMA completions get coarsened by semaphore
        #    ring reuse, so ALL dma_starts must be emitted in global
        #    deadline order — a late-needed transfer emitted early poisons
        #    the waits of critical ones.
        # 2. The shared DMA engines are bandwidth-limited early; bulk
        #    transfers issued up-front crowd out the chunk-0 criticals.
        #    So only the critical wave is issued here; the chunk 2..7
        #    trios are issued from inside the stream (gpsimd queue, which
        #    has its own semaphore pool and is otherwise idle).
        # 3. The scalar queue only carries issues that complete before the
        #    exp stream starts (it is the ACT/bottleneck queue).
        nc.sync.dma_start(wqk_sb[:], wqk[:])
        nc.scalar.dma_start(xc1[0][:], xT[128:256, 0:CH])
        nc.sync.dma_start(xc0[0][:], xT[0:128, 0:CH])
        nc.scalar.dma_start(kTt[64:128, 0:CH], onehotT[:, 0:CH])
        nc.sync.dma_start(qTt[64:128, 0:CH], bsynT[:, 0:CH])
        nc.scalar.dma_start(b2_sb[:], bias2[:])
        nc.scalar.dma_start(wv_sb[:], wv2[:])
        nc.sync.dma_start(xc0[1][:], xT[0:128, CH:2 * CH])
        nc.sync.dma_start(xc1[1][:], xT[128:256, CH:2 * CH])
        nc.sync.dma_start(qTt[64:128, CH:2 * CH], bsynT[:, CH:2 * CH])
        nc.sync.dma_start(kTt[64:128, CH:2 * CH], onehotT[:, CH:2 * CH])
        for c in range(2, NCH):
            cs = slice(c * CH, (c + 1) * CH)
            nc.sync.dma_start(xc0[c][:], xT[0:128, cs])
            nc.sync.dma_start(xc1[c][:], xT[128:256, cs])

        def emit_bias_dma(c):
            cs = slice(c * CH, (c + 1) * CH)
            nc.gpsimd.dma_start(qTt[64:128, cs], bsynT[:, cs])
            nc.gpsimd.dma_start(kTt[64:128, cs], onehotT[:, cs])

        # warm the ACT exp table (~2.7us) while projections run
        nc.scalar.activation(scr[:], b2_sb[0:1, 0:1], Exp)

        with (
            tc.tile_pool(name="psB", bufs=2, space="PSUM") as psB,
            tc.tile_pool(name="psAcc", bufs=2, space="PSUM") as psAcc,
            tc.tile_pool(name="psX", bufs=2, space="PSUM") as psX,
        ):
            # ---- helpers ----
            def emit_qk(c, pool, on_act=False):
                cs = slice(c * CH, (c + 1) * CH)
                qkp = pool.tile([128, CH], F32, name="qkp",
                                tag="oacc" if pool is psAcc else "px")
                mm(qkp[:], wqk_sb[:, 0:128], xc0[c][:], start=True, stop=False)
                mm(qkp[:], wqk_sb[:, 128:256], xc1[c][:], start=False,
                   stop=True)
                # kT eviction FIRST: during qb0 the next score group waits
                # only on kTt (qTt chunk c isn't read until qb c), so the
                # qT eviction stays off the critical chain.
                nc.vector.tensor_scalar_add(kTt[0:D, cs], qkp[D:128, :],
                                            b2_sb[D:128, 0:1])
                nc.vector.tensor_scalar_add(qTt[0:D, cs], qkp[0:D, :],
                                            b2_sb[0:D, 0:1])

            def emit_v(c):
                # v' computed DIRECTLY key-major: out[key, d] with the x
                # chunk slice as stationary and the wv half as moving (68
                # moving rows per matmul, vs 512-row projections plus PE
                # transposes). bv is folded into bo on the HOST (it only
                # shifts the normalized output by a constant per head), so
                # only 2 matmuls per key tile; the denominator ones column
                # is planted by gpsimd memsets after the eviction.
                vtr = psX.tile([128, 4, DV], F32, name="vtr", tag="px")
                for m in range(4):
                    ks = slice(m * KT, (m + 1) * KT)
                    mm(vtr[:, m:m + 1, :], xc0[c][:, ks], wv_sb[:, 0:DV],
                       start=True, stop=False)
                    mm(vtr[:, m:m + 1, :], xc1[c][:, ks], wv_sb[:, DV:2 * DV],
                       start=False, stop=True)
                nc.vector.tensor_copy(vb[:, 4 * c:4 * c + 4, :], vtr[:])
                for m in range(4):
                    j = 4 * c + m
                    nc.gpsimd.memset(vb[:, j:j + 1, LCOL:LCOL + 1], 1.0)

            oaccs = {}

            def proj_steps(qb):
                """Deferred output projection for query block qb; the oacc
                eviction happens immediately (DVE is idle), the PE matmuls
                run later in loose slots of the next block."""
                qsl = slice(qb * QB, (qb + 1) * QB)
                oacc = oaccs.pop(qb)
                nc.vector.tensor_copy(oall[:, qsl], oacc[0:D, :])
                nc.vector.tensor_copy(l_sb[:, qsl], oacc[LCOL:LCOL + 1, :])

                def s1():
                    pj = psX.tile([128, QB], F32, name="pj", tag="px")
                    ob = obp.tile([128, QB], F16, name="ob", tag="ob")
                    mm(pj[:], wo_sb[:, 0:128], oall[:, qsl],
                       start=True, stop=True)
                    nc.vector.tensor_copy(ob[:], pj[:])
                    nc.gpsimd.dma_start(outT[0:128, qsl], ob[:])

                def s2():
                    pj = psX.tile([128, QB], F32, name="pj", tag="px")
                    ob = obp.tile([128, QB], F16, name="ob", tag="ob")
                    mm(pj[:], wo_sb[:, 128:256], oall[:, qsl],
                       start=True, stop=True)
                    nc.vector.tensor_copy(ob[:], pj[:])
                    nc.gpsimd.dma_start(outT[128:256, qsl], ob[:])
                    nc.gpsimd.dma_start(lT[:, qsl], l_sb[:, qsl])

                return [s1, s2]

            def proj_last(qb):
                """Final block: same halves; casts split across Vector and
                GpSimd so they run in parallel, and the output DMAs go on
                the two hwdge queues (sync + the now-idle scalar) so no
                slow swdge drain sits at the very end."""
                qsl = slice(qb * QB, (qb + 1) * QB)
                oacc = oaccs.pop(qb)
                nc.vector.tensor_copy(oall[:, qsl], oacc[0:D, :])
                # ACT is idle once the exp stream ends: it takes the l copy
                # and the half-1 cast so both tail chains run in parallel
                # with Vector's (oacc evict + half-0 cast).
                nc.scalar.activation(l_sb[:, qsl], oacc[LCOL:LCOL + 1, :],
                                     mybir.ActivationFunctionType.Copy)
                nc.sync.dma_start(lT[:, qsl], l_sb[:, qsl])
                for half, ofs in ((0, 0), (1, 128)):
                    pj = psX.tile([128, QB], F32, name="pjl", tag="px")
                    ob = obp.tile([128, QB], F16, name="obl", tag="ob")
                    mm(pj[:], wo_sb[:, ofs:ofs + 128], oall[:, qsl],
                       start=True, stop=True)
                    if half == 0:
                        nc.vector.tensor_copy(ob[:], pj[:])
                    else:
                        nc.scalar.activation(
                            ob[:], pj[:], mybir.ActivationFunctionType.Copy)
                    q0 = qb * QB
                    eng = (nc.sync, nc.scalar)
                    eng[half].dma_start(outT[ofs:ofs + 128, q0:q0 + 256],
                                        ob[:, 0:256])
                    eng[1 - half].dma_start(
                        outT[ofs:ofs + 128, q0 + 256:q0 + 512],
                        ob[:, 256:512])

            # PE p-state warmup: dummy matmuls on the first-arrived weights
            # bridge the gap until the x chunk-0 DMA lands (so qk0 doesn't
            # run at the cold 0.65GHz p-state). A few more after qk0 keep
            # the PE busy while the DVE bias-add produces qTt/kTt chunk 0,
            # preserving the p-state ramp into the score stream.
            warm = psX.tile([128, 256], F32, name="warm", tag="px")
            for _ in range(10):
                mm(warm[:], wqk_sb[:, 0:128], wqk_sb[:], start=True, stop=True)
            emit_qk(0, psX)
            for _ in range(4):
                mm(warm[:], wqk_sb[:, 0:128], wqk_sb[:], start=True, stop=True)

            # ---- injected work, placed just ahead of each deadline:
            # kTt chunk c feeds score groups 2c..2c+1 -> qk(c) at group
            # 2c-1; vb chunk c is first read by PV(2c) which drains at
            # group 2c+PVLAG -> emit_v(c) at 2c+2. PV is lagged by a deep
            # PVLAG=10 so qb0 carries no PV work at all -> the PE (which
            # also runs all the injected projections at the not-yet-ramped
            # p-state) can keep the score stream ahead of ACT.
            inject = {
                0: [lambda: emit_bias_dma(2), lambda: emit_qk(1, psAcc)],
                1: [lambda: emit_bias_dma(3), lambda: emit_qk(2, psX)],
                2: [lambda: emit_bias_dma(4), lambda: emit_v(0)],
                3: [lambda: emit_qk(3, psAcc)],
                4: [lambda: emit_bias_dma(5), lambda: emit_v(1)],
                5: [lambda: emit_qk(4, psAcc)],
                6: [lambda: emit_bias_dma(6), lambda: emit_v(2)],
                7: [lambda: emit_qk(5, psAcc)],
                8: [lambda: emit_bias_dma(7), lambda: emit_v(3)],
                9: [lambda: emit_qk(6, psX)],
                10: [lambda: nc.gpsimd.dma_start(wo_sb[:], woT[:]),
                     lambda: emit_v(4)],
                11: [lambda: emit_qk(7, psX)],
                12: [lambda: emit_v(5)],
                14: [lambda: emit_v(6)],
            }
            inject_qb1 = {
                0: [lambda: emit_v(7)],
            }

            # ---- attention stream (PV lags scores by PVLAG groups) ----
            PVLAG = 10
            pv_queue = []
            pending_proj = []

            def emit_pv(qb, gi, p3):
                if gi == 0:
                    oaccs[qb] = psAcc.tile([DV, QB], F32, name="oacc",
                                           tag="oacc")
                oacc = oaccs[qb]
                # One fp8 DoubleRow matmul covers both key tiles of the
                # group (2x128 contraction) at 0.5 cycles/row.
                j = GRP * gi
                mm(oacc[:], vb[:, j:j + GRP, :], p3[:],
                   start=(j == 0), stop=(j == NKT - GRP),
                   perf_mode=DoubleRow)

            done_qb = {}

            def drain_one_pv():
                qb0_, gi0_, p30_ = pv_queue.pop(0)
                emit_pv(qb0_, gi0_, p30_)
                if gi0_ == NG - 1:
                    done_qb[qb0_] = True

            for qb in range(NQB):
                qsl = slice(qb * QB, (qb + 1) * QB)
                for gi in range(NG):
                    s3 = psB.tile([128, GRP, QB], F32, name="s3", tag="s3")
                    for m in range(GRP):
                        j = GRP * gi + m
                        jl = slice(j * KT, (j + 1) * KT)
                        mm(s3[:, m:m + 1, :], kTt[:, jl], qTt[:, qsl],
                           start=True, stop=True)
                    p3 = ptp.tile([128, GRP, QB], F8E4, name="p3", tag="p3")
                    # flat (depth-1) APs for the ACT engine; the 3D tile
                    # shape only matters for the DoubleRow PV operand view
                    nc.scalar.activation(p3[:].rearrange("p a b -> p (a b)"),
                                         s3[:].rearrange("p a b -> p (a b)"),
                                         Exp)
                    # injects AFTER the group's scores: the scheduler breaks
                    # readiness ties by emission order, and the static
                    # per-engine order must keep the exp stream's matmuls
                    # ahead of injected work whose DMA deps may arrive late.
                    if qb == 0:
                        for thunk in inject.get(gi, ()):
                            thunk()
                    elif qb == 1:
                        for thunk in inject_qb1.get(gi, ()):
                            thunk()
                    pv_queue.append((qb, gi, p3))
                    # Last three blocks: taper the PV lag gradually (the
                    # extra drains sit mid-block, away from the boundary
                    # where the PE p-state is still recovering) so the
                    # post-stream tail is one PV group instead of PVLAG+1.
                    if qb < NQB - 3:
                        lag = PVLAG
                    elif qb == NQB - 3:
                        lag = PVLAG - min(2, max(0, gi - 8))
                    elif qb == NQB - 2:
                        lag = 8 - min(3, max(0, gi - 5))
                    else:
                        lag = 5 - min(4, max(0, gi - 4))
                    drains = 0
                    while len(pv_queue) > lag and drains < 2:
                        drain_one_pv()
                        drains += 1
                    # previous block done accumulating? evict + start proj
                    # (extend, never replace: with the taper, done_qb can
                    # fire while the previous block's s2 is still pending)
                    if done_qb.pop(qb - 1, None):
                        pending_proj.extend(proj_steps(qb - 1))
                    if pending_proj and (gi % 6 == 5 or
                                         (qb == NQB - 1 and gi % 3 == 2)):
                        pending_proj.pop(0)()
            while pv_queue:
                drain_one_pv()
            for step in pending_proj:
                step()
            proj_last(NQB - 1)


_NC_CACHE = {}


def _get_program():
    if "nc" not in _NC_CACHE:
        _NC_CACHE["nc"] = build_program()
    return _NC_CACHE["nc"]


def make_in_maps(x, codons, syn_bias, wq, bq, wk, bk, wv, bv, wo):
    in_maps = []
    for core in range(8):
        b, h = divmod(core, NH)
        hsl = slice(h * D, (h + 1) * D)
        cod = codons[b]
        onehotT = np.zeros((D, S), np.float16)
        onehotT[cod, np.arange(S)] = 1.0
        # [wqT/8 | wkT] packed as [hi-half ; lo-half] -> [128, 256]
        wqk_full = np.concatenate([wq[hsl, :].T / 8.0, wk[hsl, :].T], axis=1)
        wqk = np.concatenate([wqk_full[0:128], wqk_full[128:256]], axis=1)
        wvp = np.concatenate(
            [wv[hsl, :].T, np.zeros((HID, DV - D), np.float32)], axis=1)
        wv2 = np.concatenate([wvp[0:128], wvp[128:256]], axis=1)
        bias2 = np.zeros((128, 2), np.float32)
        bias2[:, 0] = np.concatenate([bq[hsl] / 8.0, bk[hsl]])
        in_maps.append({
            "xT": x[b].T.astype(np.float16),
            "wqk": wqk.astype(np.float16),
            "wv2": wv2.astype(np.float16),
            "bias2": bias2,
            "bsynT": np.ascontiguousarray(syn_bias.T[:, cod]).astype(np.float16),
            "onehotT": onehotT,
            "woT": wo[:, hsl].T.astype(np.float16),
        })
    return in_maps


def kernel_run(inputs, trace=False):
    x = np.asarray(inputs["x"], np.float32)
    codons = np.asarray(inputs["codons"]).astype(np.int64)
    syn_bias = np.asarray(inputs["syn_bias"], np.float32)
    wq = np.asarray(inputs["wq"], np.float32)
    bq = np.asarray(inputs["bq"], np.float32)
    wk = np.asarray(inputs["wk"], np.float32)
    bk = np.asarray(inputs["bk"], np.float32)
    wv = np.asarray(inputs["wv"], np.float32)
    bv = np.asarray(inputs["bv"], np.float32)
    wo = np.asarray(inputs["wo"], np.float32)
    bo = np.asarray(inputs["bo"], np.float32)

    nc = _get_program()
    in_maps = make_in_maps(x, codons, syn_bias, wq, bq, wk, bk, wv, bv, wo)
    res = run_bass_kernel_spmd(nc, in_maps, core_ids=list(range(8)), trace=trace)

    # v' is computed WITHOUT bv on-device; after normalization the bias
    # contributes exactly wo @ bv per sequence position, so fold it into bo.
    bo_eff = bo + wo @ bv
    out = np.empty((B, S, HID), np.float32)
    for b in range(B):
        acc = None
        for h in range(NH):
            r = res.results[NH * b + h]
            part = r["outT"].astype(np.float32) / r["lT"]   # normalize per head
            acc = part if acc is None else acc + part
        out[b] = acc.T + bo_eff
    return out, res


def kernel(**inputs):
    out, _ = kernel_run(inputs, trace=False)
    return out



# revision 47
# speedup vs baseline: 1.0150x; 1.0057x over previous
"""CodonAttention Trainium2 kernel (fp16 stream, issue-lean pipeline).

Math (per batch b, head h):
  q = x @ wq.T + bq ; k = x @ wk.T + bk ; v = x @ wv.T + bv   (head slices)
  scores = q k^T / 8 + syn_bias[codons_i, codons_j]
  out    = softmax(scores) @ v ;  final = concat_heads(out) @ wo.T + bo

Key algebraic trick: the pairwise codon bias factors through one-hots,
  pair_bias = onehot @ syn_bias @ onehot.T
so augmenting q' = [(q+bq)/8 | onehot @ syn_bias] and k' = [k | onehot] gives
  scores = q' @ k'.T        (effective head dim 128 — exactly one partition)
Softmax runs without max-subtraction (|scores| <= ~4.3, exp safe in fp32) and
the row-sum l is obtained with a ones-column in v: [O | l] = P @ [v | 1].

Sharding: 8 cores = (batch b in {0,1}) x (head h in {0..3}). Each core runs
the full attention for its (b, h), producing the UNNORMALIZED partial
projection outT = (wo_h @ O_h.T) (256, 4096) plus denominators lT (1, 4096);
the host divides, sums the 4 head partials per batch, transposes, adds bo.

Profile-driven design (trace facts from this hardware):
- Phase B is ACT-bound: exp runs 1 col/cycle @1.2GHz regardless of dtype,
  ~1.0us per [128,1024] group; the PE streams 512-row fp16 matmuls at
  ~0.42 ns/row so 4 matmuls/group (~0.87us) fit under the exp.
- Every dma_start costs ~0.6-1us of *issue* time on its queue, so DMA
  issues are spread: Sync + Scalar queues carry x/weights (hwdge),
  GpSimd carries the bias streams and all output DMAs (swdge, idle
  engine). Queue order puts chunk 0 first so compute starts ~3us in.
- Engines execute their queue in order, so late-chunk work must not sit
  in front of the attention stream: q/k projections and v transposes
  for chunks 3..7 are injected INTO the qb-0 attention stream right
  before the groups that consume them.
- The per-block output projection is deferred two groups into the next
  query block so its oacc->oall->PE chain never stalls the score
  pipeline (it runs in loose slots, PSUM bank shared with the qk
  projection pool).
- PSUM budget (8 banks): scores double-buffer 2x2 + oacc 2 + v-flip 1 +
  qkproj/outproj shared 1.
"""

import numpy as np

import concourse.mybir as mybir
import concourse.tile as tile
from concourse import bacc
from concourse.bass_utils import run_bass_kernel_spmd


def _ensure_axon_ntff_hook():
    """This image's antenv package lacks axon_hooks, so
    run_bass_kernel_spmd(trace=True) (or BASS_TRACE=1) would die on the
    import. Register a compatible module backed by the libaxon_pjrt C ABI
    so tracing works if a caller requests it."""
    import sys
    try:
        import antenv.axon_hooks  # noqa: F401
        return
    except ImportError:
        pass
    import contextlib
    import ctypes
    import types
    try:
        lib = ctypes.CDLL("/opt/axon/libaxon_pjrt.so")
        has = hasattr(lib, "axon_start_nrt_profile")
    except OSError:
        has = False
    if has:
        lib.axon_start_nrt_profile.argtypes = [ctypes.POINTER(ctypes.c_int64),
                                               ctypes.c_size_t]
        lib.axon_start_nrt_profile.restype = ctypes.c_int64
        lib.axon_stop_nrt_profile.argtypes = [ctypes.c_char_p]
        lib.axon_stop_nrt_profile.restype = ctypes.c_int64

        @contextlib.contextmanager
        def _hook(output_dir, device_ids):
            import jax
            jax.devices()
            if device_ids:
                ids = (ctypes.c_int64 * len(device_ids))(*device_ids)
                rc = lib.axon_start_nrt_profile(ids, len(device_ids))
            else:
                rc = lib.axon_start_nrt_profile(None, 0)
            if rc != 0:
                raise RuntimeError(f"axon_start_nrt_profile rc={rc}")
            try:
                yield
            finally:
                lib.axon_stop_nrt_profile(str(output_dir).encode())
    else:
        _hook = None

    mod = types.ModuleType("antenv.axon_hooks")
    _state = {"hook": _hook}
    mod.get_axon_ntff_profile_hook = lambda: _state["hook"]
    mod.set_axon_ntff_profile_hook = lambda h: _state.__setitem__("hook", h)
    sys.modules["antenv.axon_hooks"] = mod


_ensure_axon_ntff_hook()

B, S, HID, NH, D = 2, 4096, 256, 4, 64
DV = D + 16        # v cols + ones col + pad: DoubleRow requires the k-tile
                   # stride (outermost lhsT free step) to be 16B-aligned
LCOL = D           # index of the ones column inside a v tile
QB = 512           # query block (free dim of score matmuls)
KT = 128           # key tile (partition dim of transposed scores)
CH = 512           # x chunk width
NCH = S // CH      # 8
NQB = S // QB      # 8
NKT = S // KT      # 32
GRP = 2            # key tiles per exp group (2 PSUM banks per group)
NG = NKT // GRP    # 16 groups per query block

F32 = mybir.dt.float32
F32R = mybir.dt.float32r
F16 = mybir.dt.float16
F8E4 = mybir.dt.float8e4
Exp = mybir.ActivationFunctionType.Exp
DoubleRow = mybir.MatmulPerfMode.DoubleRow


def build_program():
    nc = bacc.Bacc("TRN2", target_bir_lowering=False, debug=False, num_devices=8)

    def di(name, shape, dt=F16):
        return nc.dram_tensor(name, shape, dt, kind="ExternalInput").ap()

    xT = di("xT", [HID, S])            # x[b].T
    wqk = di("wqk", [128, 256])        # [wqT_hi/8|wkT_hi ; wqT_lo/8|wkT_lo]
    wv2 = di("wv2", [128, 2 * DV])     # [wvT_hi | wvT_lo], col 64.. pad 0
    bias2 = di("bias2", [128, 2], F32) # col0 = [bq/8; bk]
    bsynT = di("bsynT", [D, S])        # (onehot @ syn_bias).T
    onehotT = di("onehotT", [D, S])
    woT = di("woT", [D, HID])          # wo[:, hslice].T
    outT = nc.dram_tensor("outT", [HID, S], F16, kind="ExternalOutput").ap()
    lT = nc.dram_tensor("lT", [1, S], F32, kind="ExternalOutput").ap()

    with tile.TileContext(nc) as tc:
        _body(tc, xT, wqk, wv2, bias2, bsynT, onehotT, woT, outT, lT)
    nc.compile()
    return nc


def _body(tc, xT, wqk, wv2, bias2, bsynT, onehotT, woT, outT, lT):
    nc = tc.nc
    mm = nc.tensor.matmul

    with (
        tc.tile_pool(name="const", bufs=1) as constp,
        tc.tile_pool(name="big", bufs=1) as bigp,
        tc.tile_pool(name="pt", bufs=12) as ptp,
        tc.tile_pool(name="ob", bufs=2) as obp,
    ):
        # ---- constants ----
        wqk_sb = constp.tile([128, 256], F16, name="wqk_sb", tag="wqk_sb")
        wv_sb = constp.tile([128, 2 * DV], F16, name="wv_sb", tag="wv_sb")
        b2_sb = constp.tile([128, 2], F32, name="b2_sb", tag="b2_sb")
        wo_sb = constp.tile([D, HID], F16, name="wo_sb", tag="wo_sb")
        scr = constp.tile([1, 1], F32, name="scr", tag="scr")

        # persistent activations (subregion deps make slices per-chunk)
        xc0 = [bigp.tile([128, CH], F16, name=f"xc0_{c}", tag=f"xc0_{c}")
               for c in range(NCH)]
        xc1 = [bigp.tile([128, CH], F16, name=f"xc1_{c}", tag=f"xc1_{c}")
               for c in range(NCH)]
        qTt = bigp.tile([128, S], F16, name="qTt", tag="qTt")  # 0:64 q/8, 64:128 bsynT
        kTt = bigp.tile([128, S], F16, name="kTt", tag="kTt")  # 0:64 k,   64:128 onehotT
        # v' key-major, fp8 (PV runs in DoubleRow perf mode: 2 key tiles
        # per matmul at 0.5 cycles/row). [partition=key, ktile, dv]
        vb = bigp.tile([128, NKT, DV], F8E4, name="vb", tag="vb")
        oall = bigp.tile([D, S], F16, name="oall", tag="oall")
        l_sb = bigp.tile([1, S], F32, name="l_sb", tag="l_sb")

        # ---- DMA issues. Rules learned from traces:
        # 1. Dependencies on DMA completions get coarsened by semaphore
        #    ring reuse, so ALL dma_starts must be emitted in global
        #    deadline order — a late-needed transfer emitted early poisons
        #    the waits of critical ones.
        # 2. The shared DMA engines are bandwidth-limited early; bulk
        #    transfers issued up-front crowd out the chunk-0 criticals.
        #    So only the critical wave is issued here; the chunk 2..7
        #    trios are issued from inside the stream (gpsimd queue, which
        #    has its own semaphore pool and is otherwise idle).
        # 3. The scalar queue only carries issues that complete before the
        #    exp stream starts (it is the ACT/bottleneck queue).
        nc.sync.dma_start(wqk_sb[:], wqk[:])
        nc.scalar.dma_start(xc1[0][:], xT[128:256, 0:CH])
        nc.sync.dma_start(xc0[0][:], xT[0:128, 0:CH])
        nc.scalar.dma_start(kTt[64:128, 0:CH], onehotT[:, 0:CH])
        nc.sync.dma_start(qTt[64:128, 0:CH], bsynT[:, 0:CH])
        nc.scalar.dma_start(b2_sb[:], bias2[:])
        nc.scalar.dma_start(wv_sb[:], wv2[:])
        nc.sync.dma_start(xc0[1][:], xT[0:128, CH:2 * CH])
        nc.sync.dma_start(xc1[1][:], xT[128:256, CH:2 * CH])
        nc.sync.dma_start(qTt[64:128, CH:2 * CH], bsynT[:, CH:2 * CH])
        nc.sync.dma_start(kTt[64:128, CH:2 * CH], onehotT[:, CH:2 * CH])
        for c in range(2, NCH):
            cs = slice(c * CH, (c + 1) * CH)
            nc.sync.dma_start(xc0[c][:], xT[0:128, cs])
            nc.sync.dma_start(xc1[c][:], xT[128:256, cs])

        def emit_bias_dma(c):
            cs = slice(c * CH, (c + 1) * CH)
            nc.gpsimd.dma_start(qTt[64:128, cs], bsynT[:, cs])
            nc.gpsimd.dma_start(kTt[64:128, cs], onehotT[:, cs])

        # warm the ACT exp table (~2.7us) while projections run
        nc.scalar.activation(scr[:], b2_sb[0:1, 0:1], Exp)

        with (
            tc.tile_pool(name="psB", bufs=2, space="PSUM") as psB,
            tc.tile_pool(name="psAcc", bufs=2, space="PSUM") as psAcc,
            tc.tile_pool(name="psX", bufs=2, space="PSUM") as psX,
        ):
            # ---- helpers ----
            Ident = mybir.ActivationFunctionType.Identity

            def emit_qk(c, pool, on_act=False):
                cs = slice(c * CH, (c + 1) * CH)
                qkp = pool.tile([128, CH], F32, name="qkp",
                                tag="oacc" if pool is psAcc else "px")
                mm(qkp[:], wqk_sb[:, 0:128], xc0[c][:], start=True, stop=False)
                mm(qkp[:], wqk_sb[:, 128:256], xc1[c][:], start=False,
                   stop=True)
                # kT eviction FIRST: during qb0 the next score group waits
                # only on kTt (qTt chunk c isn't read until qb c), so the
                # qT eviction stays off the critical chain. For chunk 0 the
                # evictions run on the ACT engine (idle until the first exp,
                # ~600ns each vs ~780ns on DVE, and off the DVE latency
                # chain); Identity shares the exp act table, so no reload.
                if on_act:
                    nc.scalar.activation(kTt[0:D, cs], qkp[D:128, :], Ident,
                                         bias=b2_sb[D:128, 0:1])
                    nc.scalar.activation(qTt[0:D, cs], qkp[0:D, :], Ident,
                                         bias=b2_sb[0:D, 0:1])
                else:
                    nc.vector.tensor_scalar_add(kTt[0:D, cs], qkp[D:128, :],
                                                b2_sb[D:128, 0:1])
                    nc.vector.tensor_scalar_add(qTt[0:D, cs], qkp[0:D, :],
                                                b2_sb[0:D, 0:1])

            def emit_v(c):
                # v' computed DIRECTLY key-major: out[key, d] with the x
                # chunk slice as stationary and the wv half as moving (68
                # moving rows per matmul, vs 512-row projections plus PE
                # transposes). bv is folded into bo on the HOST (it only
                # shifts the normalized output by a constant per head), so
                # only 2 matmuls per key tile; the denominator ones column
                # is planted by gpsimd memsets after the eviction.
                vtr = psX.tile([128, 4, DV], F32, name="vtr", tag="px")
                for m in range(4):
                    ks = slice(m * KT, (m + 1) * KT)
                    mm(vtr[:, m:m + 1, :], xc0[c][:, ks], wv_sb[:, 0:DV],
                       start=True, stop=False)
                    mm(vtr[:, m:m + 1, :], xc1[c][:, ks], wv_sb[:, DV:2 * DV],
                       start=False, stop=True)
                nc.vector.tensor_copy(vb[:, 4 * c:4 * c + 4, :], vtr[:])
                for m in range(4):
                    j = 4 * c + m
                    nc.gpsimd.memset(vb[:, j:j + 1, LCOL:LCOL + 1], 1.0)

            oaccs = {}

            def proj_steps(qb):
                """Deferred output projection for query block qb; the oacc
                eviction happens immediately (DVE is idle), the PE matmuls
                run later in loose slots of the next block."""
                qsl = slice(qb * QB, (qb + 1) * QB)
                oacc = oaccs.pop(qb)
                nc.vector.tensor_copy(oall[:, qsl], oacc[0:D, :])
                nc.vector.tensor_copy(l_sb[:, qsl], oacc[LCOL:LCOL + 1, :])

                def s1():
                    pj = psX.tile([128, QB], F32, name="pj", tag="px")
                    ob = obp.tile([128, QB], F16, name="ob", tag="ob")
                    mm(pj[:], wo_sb[:, 0:128], oall[:, qsl],
                       start=True, stop=True)
                    nc.vector.tensor_copy(ob[:], pj[:])
                    nc.gpsimd.dma_start(outT[0:128, qsl], ob[:])

                def s2():
                    pj = psX.tile([128, QB], F32, name="pj", tag="px")
                    ob = obp.tile([128, QB], F16, name="ob", tag="ob")
                    mm(pj[:], wo_sb[:, 128:256], oall[:, qsl],
                       start=True, stop=True)
                    nc.vector.tensor_copy(ob[:], pj[:])
                    nc.gpsimd.dma_start(outT[128:256, qsl], ob[:])
                    nc.gpsimd.dma_start(lT[:, qsl], l_sb[:, qsl])

                return [s1, s2]

            def proj_last(qb):
                """Final block: same halves; casts split across Vector and
                GpSimd so they run in parallel, and the output DMAs go on
                the two hwdge queues (sync + the now-idle scalar) so no
                slow swdge drain sits at the very end."""
                qsl = slice(qb * QB, (qb + 1) * QB)
                oacc = oaccs.pop(qb)
                nc.vector.tensor_copy(oall[:, qsl], oacc[0:D, :])
                # ACT is idle once the exp stream ends: it takes the l copy
                # and the half-1 cast so both tail chains run in parallel
                # with Vector's (oacc evict + half-0 cast).
                nc.scalar.activation(l_sb[:, qsl], oacc[LCOL:LCOL + 1, :],
                                     mybir.ActivationFunctionType.Copy)
                nc.sync.dma_start(lT[:, qsl], l_sb[:, qsl])
                for half, ofs in ((0, 0), (1, 128)):
                    pj = psX.tile([128, QB], F32, name="pjl", tag="px")
                    ob = obp.tile([128, QB], F16, name="obl", tag="ob")
                    mm(pj[:], wo_sb[:, ofs:ofs + 128], oall[:, qsl],
                       start=True, stop=True)
                    if half == 0:
                        nc.vector.tensor_copy(ob[:], pj[:])
                    else:
                        nc.scalar.activation(
                            ob[:], pj[:], mybir.ActivationFunctionType.Copy)
                    q0 = qb * QB
                    eng = (nc.sync, nc.scalar)
                    eng[half].dma_start(outT[ofs:ofs + 128, q0:q0 + 256],
                                        ob[:, 0:256])
                    eng[1 - half].dma_start(
                        outT[ofs:ofs + 128, q0 + 256:q0 + 512],
                        ob[:, 256:512])

            # PE p-state warmup: dummy matmuls on the first-arrived weights
            # bridge the gap until the x chunk-0 DMA lands (so qk0 doesn't
            # run at the cold 0.65GHz p-state). A few more after qk0 keep
            # the PE busy while the DVE bias-add produces qTt/kTt chunk 0,
            # preserving the p-state ramp into the score stream.
            warm = psX.tile([128, 256], F32, name="warm", tag="px")
            for _ in range(10):
                mm(warm[:], wqk_sb[:, 0:128], wqk_sb[:], start=True, stop=True)
            emit_qk(0, psX, on_act=True)
            for _ in range(4):
                mm(warm[:], wqk_sb[:, 0:128], wqk_sb[:], start=True, stop=True)

            # ---- injected work, placed just ahead of each deadline:
            # kTt chunk c feeds score groups 2c..2c+1 -> qk(c) at group
            # 2c-1; vb chunk c is first read by PV(2c) which drains at
            # group 2c+PVLAG -> emit_v(c) at 2c+2. PV is lagged by a deep
            # PVLAG=10 so qb0 carries no PV work at all -> the PE (which
            # also runs all the injected projections at the not-yet-ramped
            # p-state) can keep the score stream ahead of ACT.
            inject = {
                0: [lambda: emit_bias_dma(2)],
                1: [lambda: emit_bias_dma(3), lambda: emit_qk(1, psAcc)],
                2: [lambda: emit_bias_dma(4), lambda: emit_v(0)],
                3: [lambda: emit_qk(2, psX)],
                4: [lambda: emit_bias_dma(5), lambda: emit_v(1)],
                5: [lambda: emit_qk(3, psAcc)],
                6: [lambda: emit_bias_dma(6), lambda: emit_v(2)],
                7: [lambda: emit_qk(4, psAcc)],
                8: [lambda: emit_bias_dma(7), lambda: emit_v(3)],
                9: [lambda: emit_qk(5, psAcc)],
                10: [lambda: nc.gpsimd.dma_start(wo_sb[:], woT[:]),
                     lambda: emit_v(4)],
                11: [lambda: emit_qk(6, psX)],
                12: [lambda: emit_v(5)],
                13: [lambda: emit_qk(7, psX)],
                14: [lambda: emit_v(6)],
            }
            inject_qb1 = {
                0: [lambda: emit_v(7)],
            }

            # ---- attention stream (PV lags scores by PVLAG groups) ----
            PVLAG = 10
            pv_queue = []
            pending_proj = []

            def emit_pv(qb, gi, p3):
                if gi == 0:
                    oaccs[qb] = psAcc.tile([DV, QB], F32, name="oacc",
                                           tag="oacc")
                oacc = oaccs[qb]
                # One fp8 DoubleRow matmul covers both key tiles of the
                # group (2x128 contraction) at 0.5 cycles/row.
                j = GRP * gi
                mm(oacc[:], vb[:, j:j + GRP, :], p3[:],
                   start=(j == 0), stop=(j == NKT - GRP),
                   perf_mode=DoubleRow)

            done_qb = {}

            def drain_one_pv():
                qb0_, gi0_, p30_ = pv_queue.pop(0)
                emit_pv(qb0_, gi0_, p30_)
                if gi0_ == NG - 1:
                    done_qb[qb0_] = True

            for qb in range(NQB):
                qsl = slice(qb * QB, (qb + 1) * QB)
                for gi in range(NG):
                    if qb == 0:
                        for thunk in inject.get(gi, ()):
                            thunk()
                    elif qb == 1:
                        for thunk in inject_qb1.get(gi, ()):
                            thunk()
                    s3 = psB.tile([128, GRP, QB], F32, name="s3", tag="s3")
                    for m in range(GRP):
                        j = GRP * gi + m
                        jl = slice(j * KT, (j + 1) * KT)
                        mm(s3[:, m:m + 1, :], kTt[:, jl], qTt[:, qsl],
                           start=True, stop=True)
                    p3 = ptp.tile([128, GRP, QB], F8E4, name="p3", tag="p3")
                    # flat (depth-1) APs for the ACT engine; the 3D tile
                    # shape only matters for the DoubleRow PV operand view
                    nc.scalar.activation(p3[:].rearrange("p a b -> p (a b)"),
                                         s3[:].rearrange("p a b -> p (a b)"),
                                         Exp)
                    pv_queue.append((qb, gi, p3))
                    # Last three blocks: taper the PV lag gradually (the
                    # extra drains sit mid-block, away from the boundary
                    # where the PE p-state is still recovering) so the
                    # post-stream tail is one PV group instead of PVLAG+1.
                    if qb < NQB - 3:
                        lag = PVLAG
                    elif qb == NQB - 3:
                        lag = PVLAG - min(2, max(0, gi - 8))
                    elif qb == NQB - 2:
                        lag = 8 - min(3, max(0, gi - 5))
                    else:
                        lag = 5 - min(4, max(0, gi - 4))
                    drains = 0
                    while len(pv_queue) > lag and drains < 2:
                        drain_one_pv()
                        drains += 1
                    # previous block done accumulating? evict + start proj
                    # (extend, never replace: with the taper, done_qb can
                    # fire while the previous block's s2 is still pending)
                    if done_qb.pop(qb - 1, None):
                        pending_proj.extend(proj_steps(qb - 1))
                    if pending_proj and (gi % 6 == 5 or
                                         (qb == NQB - 1 and gi % 3 == 2)):
                        pending_proj.pop(0)()
            while pv_queue:
                drain_one_pv()
            for step in pending_proj:
                step()
            proj_last(NQB - 1)


_NC_CACHE = {}


def _get_program():
    if "nc" not in _NC_CACHE:
        _NC_CACHE["nc"] = build_program()
    return _NC_CACHE["nc"]


def make_in_maps(x, codons, syn_bias, wq, bq, wk, bk, wv, bv, wo):
    in_maps = []
    for core in range(8):
        b, h = divmod(core, NH)
        hsl = slice(h * D, (h + 1) * D)
        cod = codons[b]
        onehotT = np.zeros((D, S), np.float16)
        onehotT[cod, np.arange(S)] = 1.0
        # [wqT/8 | wkT] packed as [hi-half ; lo-half] -> [128, 256]
        wqk_full = np.concatenate([wq[hsl, :].T / 8.0, wk[hsl, :].T], axis=1)
        wqk = np.concatenate([wqk_full[0:128], wqk_full[128:256]], axis=1)
        wvp = np.concatenate(
            [wv[hsl, :].T, np.zeros((HID, DV - D), np.float32)], axis=1)
        wv2 = np.concatenate([wvp[0:128], wvp[128:256]], axis=1)
        bias2 = np.zeros((128, 2), np.float32)
        bias2[:, 0] = np.concatenate([bq[hsl] / 8.0, bk[hsl]])
        in_maps.append({
            "xT": x[b].T.astype(np.float16),
            "wqk": wqk.astype(np.float16),
            "wv2": wv2.astype(np.float16),
            "bias2": bias2,
            "bsynT": np.ascontiguousarray(syn_bias.T[:, cod]).astype(np.float16),
            "onehotT": onehotT,
            "woT": wo[:, hsl].T.astype(np.float16),
        })
    return in_maps


def kernel_run(inputs, trace=False):
    x = np.asarray(inputs["x"], np.float32)
    codons = np.asarray(inputs["codons"]).astype(np.int64)
    syn_bias = np.asarray(inputs["syn_bias"], np.float32)
    wq = np.asarray(inputs["wq"], np.float32)
    bq = np.asarray(inputs["bq"], np.float32)
    wk = np.asarray(inputs["wk"], np.float32)
    bk = np.asarray(inputs["bk"], np.float32)
    wv = np.asarray(inputs["wv"], np.float32)
    bv = np.asarray(inputs["bv"], np.float32)
    wo = np.asarray(inputs["wo"], np.float32)
    bo = np.asarray(inputs["bo"], np.float32)

    nc = _get_program()
    in_maps = make_in_maps(x, codons, syn_bias, wq, bq, wk, bk, wv, bv, wo)
    res = run_bass_kernel_spmd(nc, in_maps, core_ids=list(range(8)), trace=trace)

    # v' is computed WITHOUT bv on-device; after normalization the bias
    # contributes exactly wo @ bv per sequence position, so fold it into bo.
    bo_eff = bo + wo @ bv
    out = np.empty((B, S, HID), np.float32)
    for b in range(B):
        acc = None
        for h in range(NH):
            r = res.results[NH * b + h]
            part = r["outT"].astype(np.float32) / r["lT"]   # normalize per head
            acc = part if acc is None else acc + part
        out[b] = acc.T + bo_eff
    return out, res


def kernel(**inputs):
    out, _ = kernel_run(inputs, trace=False)
    return out



# revision 53
# speedup vs baseline: 1.0384x; 1.0231x over previous
"""CodonAttention Trainium2 kernel (fp16 stream, issue-lean pipeline).

Math (per batch b, head h):
  q = x @ wq.T + bq ; k = x @ wk.T + bk ; v = x @ wv.T + bv   (head slices)
  scores = q k^T / 8 + syn_bias[codons_i, codons_j]
  out    = softmax(scores) @ v ;  final = concat_heads(out) @ wo.T + bo

Key algebraic trick: the pairwise codon bias factors through one-hots,
  pair_bias = onehot @ syn_bias @ onehot.T
so augmenting q' = [(q+bq)/8 | onehot @ syn_bias] and k' = [k | onehot] gives
  scores = q' @ k'.T        (effective head dim 128 — exactly one partition)
Softmax runs without max-subtraction (|scores| <= ~4.3, exp safe in fp32) and
the row-sum l is obtained with a ones-column in v: [O | l] = P @ [v | 1].

Sharding: 8 cores = (batch b in {0,1}) x (head h in {0..3}). Each core runs
the full attention for its (b, h), producing the UNNORMALIZED partial
projection outT = (wo_h @ O_h.T) (256, 4096) plus denominators lT (1, 4096);
the host divides, sums the 4 head partials per batch, transposes, adds bo.

Profile-driven design (trace facts from this hardware):
- Phase B is ACT-bound: exp runs 1 col/cycle @1.2GHz regardless of dtype,
  ~1.0us per [128,1024] group; the PE streams 512-row fp16 matmuls at
  ~0.42 ns/row so 4 matmuls/group (~0.87us) fit under the exp.
- Every dma_start costs ~0.6-1us of *issue* time on its queue, so DMA
  issues are spread: Sync + Scalar queues carry x/weights (hwdge),
  GpSimd carries the bias streams and all output DMAs (swdge, idle
  engine). Queue order puts chunk 0 first so compute starts ~3us in.
- Engines execute their queue in order, so late-chunk work must not sit
  in front of the attention stream: q/k projections and v transposes
  for chunks 3..7 are injected INTO the qb-0 attention stream right
  before the groups that consume them.
- The per-block output projection is deferred two groups into the next
  query block so its oacc->oall->PE chain never stalls the score
  pipeline (it runs in loose slots, PSUM bank shared with the qk
  projection pool).
- PSUM budget (8 banks): scores double-buffer 2x2 + oacc 2 + v-flip 1 +
  qkproj/outproj shared 1.
"""

import numpy as np

import concourse.mybir as mybir
import concourse.tile as tile
from concourse import bacc
from concourse.bass_utils import run_bass_kernel_spmd


def _ensure_axon_ntff_hook():
    """This image's antenv package lacks axon_hooks, so
    run_bass_kernel_spmd(trace=True) (or BASS_TRACE=1) would die on the
    import. Register a compatible module backed by the libaxon_pjrt C ABI
    so tracing works if a caller requests it."""
    import sys
    try:
        import antenv.axon_hooks  # noqa: F401
        return
    except ImportError:
        pass
    import contextlib
    import ctypes
    import types
    try:
        lib = ctypes.CDLL("/opt/axon/libaxon_pjrt.so")
        has = hasattr(lib, "axon_start_nrt_profile")
    except OSError:
        has = False
    if has:
        lib.axon_start_nrt_profile.argtypes = [ctypes.POINTER(ctypes.c_int64),
                                               ctypes.c_size_t]
        lib.axon_start_nrt_profile.restype = ctypes.c_int64
        lib.axon_stop_nrt_profile.argtypes = [ctypes.c_char_p]
        lib.axon_stop_nrt_profile.restype = ctypes.c_int64

        @contextlib.contextmanager
        def _hook(output_dir, device_ids):
            import jax
            jax.devices()
            if device_ids:
                ids = (ctypes.c_int64 * len(device_ids))(*device_ids)
                rc = lib.axon_start_nrt_profile(ids, len(device_ids))
            else:
                rc = lib.axon_start_nrt_profile(None, 0)
            if rc != 0:
                raise RuntimeError(f"axon_start_nrt_profile rc={rc}")
            try:
                yield
            finally:
                lib.axon_stop_nrt_profile(str(output_dir).encode())
    else:
        _hook = None

    mod = types.ModuleType("antenv.axon_hooks")
    _state = {"hook": _hook}
    mod.get_axon_ntff_profile_hook = lambda: _state["hook"]
    mod.set_axon_ntff_profile_hook = lambda h: _state.__setitem__("hook", h)
    sys.modules["antenv.axon_hooks"] = mod


_ensure_axon_ntff_hook()

B, S, HID, NH, D = 2, 4096, 256, 4, 64
DV = D + 16        # v cols + ones col + pad: DoubleRow requires the k-tile
                   # stride (outermost lhsT free step) to be 16B-aligned
LCOL = D           # index of the ones column inside a v tile
QB = 512           # query block (free dim of score matmuls)
KT = 128           # key tile (partition dim of transposed scores)
CH = 512           # x chunk width
NCH = S // CH      # 8
NQB = S // QB      # 8
NKT = S // KT      # 32
GRP = 3            # key tiles per exp group (3 PSUM banks per group);
                   # bigger groups amortize the ~330ns/instr ACT overhead
NG = (NKT + GRP - 1) // GRP   # 11 groups per query block (last has 2)


def group_tiles(gi):
    j0 = GRP * gi
    return j0, min(GRP, NKT - j0)

F32 = mybir.dt.float32
F32R = mybir.dt.float32r
F16 = mybir.dt.float16
F8E4 = mybir.dt.float8e4
Exp = mybir.ActivationFunctionType.Exp
DoubleRow = mybir.MatmulPerfMode.DoubleRow


def build_program():
    nc = bacc.Bacc("TRN2", target_bir_lowering=False, debug=False, num_devices=8)

    def di(name, shape, dt=F16):
        return nc.dram_tensor(name, shape, dt, kind="ExternalInput").ap()

    xT = di("xT", [HID, S])            # x[b].T
    wqk = di("wqk", [128, 256])        # [wqT_hi/8|wkT_hi ; wqT_lo/8|wkT_lo]
    wv2 = di("wv2", [128, 2 * DV])     # [wvT_hi | wvT_lo], col 64.. pad 0
    bias2 = di("bias2", [128, 2], F32) # col0 = [bq/8; bk]
    bsynT = di("bsynT", [D, S])        # (onehot @ syn_bias).T
    onehotT = di("onehotT", [D, S])
    woT = di("woT", [D, HID])          # wo[:, hslice].T
    outT = nc.dram_tensor("outT", [HID, S], F16, kind="ExternalOutput").ap()
    lT = nc.dram_tensor("lT", [1, S], F32, kind="ExternalOutput").ap()

    with tile.TileContext(nc) as tc:
        _body(tc, xT, wqk, wv2, bias2, bsynT, onehotT, woT, outT, lT)
    nc.compile()
    return nc


def _body(tc, xT, wqk, wv2, bias2, bsynT, onehotT, woT, outT, lT):
    nc = tc.nc
    mm = nc.tensor.matmul

    with (
        tc.tile_pool(name="const", bufs=1) as constp,
        tc.tile_pool(name="big", bufs=1) as bigp,
        tc.tile_pool(name="pt", bufs=12) as ptp,
        tc.tile_pool(name="ob", bufs=2) as obp,
    ):
        # ---- constants ----
        wqk_sb = constp.tile([128, 256], F16, name="wqk_sb", tag="wqk_sb")
        wv_sb = constp.tile([128, 2 * DV], F16, name="wv_sb", tag="wv_sb")
        b2_sb = constp.tile([128, 2], F32, name="b2_sb", tag="b2_sb")
        wo_sb = constp.tile([D, HID], F16, name="wo_sb", tag="wo_sb")
        scr = constp.tile([1, 1], F32, name="scr", tag="scr")

        # persistent activations (subregion deps make slices per-chunk)
        xc0 = [bigp.tile([128, CH], F16, name=f"xc0_{c}", tag=f"xc0_{c}")
               for c in range(NCH)]
        xc1 = [bigp.tile([128, CH], F16, name=f"xc1_{c}", tag=f"xc1_{c}")
               for c in range(NCH)]
        qTt = bigp.tile([128, S], F16, name="qTt", tag="qTt")  # 0:64 q/8, 64:128 bsynT
        kTt = bigp.tile([128, S], F16, name="kTt", tag="kTt")  # 0:64 k,   64:128 onehotT
        # v' key-major, fp8 (PV runs in DoubleRow perf mode: 2 key tiles
        # per matmul at 0.5 cycles/row). [partition=key, ktile, dv]
        vb = bigp.tile([128, NKT, DV], F8E4, name="vb", tag="vb")
        oall = bigp.tile([D, S], F16, name="oall", tag="oall")
        l_sb = bigp.tile([1, S], F32, name="l_sb", tag="l_sb")

        # ---- DMA issues. Rules learned from traces:
        # 1. Dependencies on DMA completions get coarsened by semaphore
        #    ring reuse, so ALL dma_starts must be emitted in global
        #    deadline order — a late-needed transfer emitted early poisons
        #    the waits of critical ones.
        # 2. The shared DMA engines are bandwidth-limited early; bulk
        #    transfers issued up-front crowd out the chunk-0 criticals.
        #    So only the critical wave is issued here; the chunk 2..7
        #    trios are issued from inside the stream (gpsimd queue, which
        #    has its own semaphore pool and is otherwise idle).
        # 3. The scalar queue only carries issues that complete before the
        #    exp stream starts (it is the ACT/bottleneck queue).
        nc.sync.dma_start(wqk_sb[:], wqk[:])
        nc.scalar.dma_start(xc1[0][:], xT[128:256, 0:CH])
        nc.sync.dma_start(xc0[0][:], xT[0:128, 0:CH])
        nc.scalar.dma_start(kTt[64:128, 0:CH], onehotT[:, 0:CH])
        nc.sync.dma_start(qTt[64:128, 0:CH], bsynT[:, 0:CH])
        nc.scalar.dma_start(b2_sb[:], bias2[:])
        nc.scalar.dma_start(wv_sb[:], wv2[:])
        nc.sync.dma_start(xc0[1][:], xT[0:128, CH:2 * CH])
        nc.sync.dma_start(xc1[1][:], xT[128:256, CH:2 * CH])
        nc.sync.dma_start(qTt[64:128, CH:2 * CH], bsynT[:, CH:2 * CH])
        nc.sync.dma_start(kTt[64:128, CH:2 * CH], onehotT[:, CH:2 * CH])
        for c in range(2, NCH):
            cs = slice(c * CH, (c + 1) * CH)
            nc.sync.dma_start(xc0[c][:], xT[0:128, cs])
            nc.sync.dma_start(xc1[c][:], xT[128:256, cs])

        def emit_bias_dma(c):
            cs = slice(c * CH, (c + 1) * CH)
            nc.gpsimd.dma_start(qTt[64:128, cs], bsynT[:, cs])
            nc.gpsimd.dma_start(kTt[64:128, cs], onehotT[:, cs])

        # warm the ACT exp table (~2.7us) while projections run
        nc.scalar.activation(scr[:], b2_sb[0:1, 0:1], Exp)

        with (
            tc.tile_pool(name="psB", bufs=2, space="PSUM") as psB,
            tc.tile_pool(name="psAcc", bufs=1, space="PSUM") as psAcc,
            tc.tile_pool(name="psX", bufs=1, space="PSUM") as psX,
        ):
            # ---- helpers ----
            Ident = mybir.ActivationFunctionType.Identity

            def emit_qk(c, pool, on_act=False):
                cs = slice(c * CH, (c + 1) * CH)
                qkp = pool.tile([128, CH], F32, name="qkp",
                                tag="oacc" if pool is psAcc else "px")
                mm(qkp[:], wqk_sb[:, 0:128], xc0[c][:], start=True, stop=False)
                mm(qkp[:], wqk_sb[:, 128:256], xc1[c][:], start=False,
                   stop=True)
                # kT eviction FIRST: during qb0 the next score group waits
                # only on kTt (qTt chunk c isn't read until qb c), so the
                # qT eviction stays off the critical chain. For chunk 0 the
                # evictions run on the ACT engine (idle until the first exp,
                # ~600ns each vs ~780ns on DVE, and off the DVE latency
                # chain); Identity shares the exp act table, so no reload.
                if on_act:
                    nc.scalar.activation(kTt[0:D, cs], qkp[D:128, :], Ident,
                                         bias=b2_sb[D:128, 0:1])
                    nc.scalar.activation(qTt[0:D, cs], qkp[0:D, :], Ident,
                                         bias=b2_sb[0:D, 0:1])
                else:
                    nc.vector.tensor_scalar_add(kTt[0:D, cs], qkp[D:128, :],
                                                b2_sb[D:128, 0:1])
                    nc.vector.tensor_scalar_add(qTt[0:D, cs], qkp[0:D, :],
                                                b2_sb[0:D, 0:1])

            def emit_v(c):
                # v' computed DIRECTLY key-major: out[key, d] with the x
                # chunk slice as stationary and the wv half as moving (68
                # moving rows per matmul, vs 512-row projections plus PE
                # transposes). bv is folded into bo on the HOST (it only
                # shifts the normalized output by a constant per head), so
                # only 2 matmuls per key tile; the denominator ones column
                # is planted by gpsimd memsets after the eviction.
                vtr = psX.tile([128, 4, DV], F32, name="vtr", tag="px")
                for m in range(4):
                    ks = slice(m * KT, (m + 1) * KT)
                    mm(vtr[:, m:m + 1, :], xc0[c][:, ks], wv_sb[:, 0:DV],
                       start=True, stop=False)
                    mm(vtr[:, m:m + 1, :], xc1[c][:, ks], wv_sb[:, DV:2 * DV],
                       start=False, stop=True)
                nc.vector.tensor_copy(vb[:, 4 * c:4 * c + 4, :], vtr[:])
                for m in range(4):
                    j = 4 * c + m
                    nc.gpsimd.memset(vb[:, j:j + 1, LCOL:LCOL + 1], 1.0)

            oaccs = {}

            def proj_steps(qb):
                """Deferred output projection for query block qb; the oacc
                eviction happens immediately (DVE is idle), the PE matmuls
                run later in loose slots of the next block."""
                qsl = slice(qb * QB, (qb + 1) * QB)
                oacc = oaccs.pop(qb)
                nc.vector.tensor_copy(oall[:, qsl], oacc[0:D, :])
                nc.vector.tensor_copy(l_sb[:, qsl], oacc[LCOL:LCOL + 1, :])

                def s1():
                    pj = psX.tile([128, QB], F32, name="pj", tag="px")
                    ob = obp.tile([128, QB], F16, name="ob", tag="ob")
                    mm(pj[:], wo_sb[:, 0:128], oall[:, qsl],
                       start=True, stop=True)
                    nc.vector.tensor_copy(ob[:], pj[:])
                    nc.gpsimd.dma_start(outT[0:128, qsl], ob[:])

                def s2():
                    pj = psX.tile([128, QB], F32, name="pj", tag="px")
                    ob = obp.tile([128, QB], F16, name="ob", tag="ob")
                    mm(pj[:], wo_sb[:, 128:256], oall[:, qsl],
                       start=True, stop=True)
                    nc.vector.tensor_copy(ob[:], pj[:])
                    nc.gpsimd.dma_start(outT[128:256, qsl], ob[:])
                    nc.gpsimd.dma_start(lT[:, qsl], l_sb[:, qsl])

                return [s1, s2]

            def proj_last(qb):
                """Final block: same halves; casts split across Vector and
                GpSimd so they run in parallel, and the output DMAs go on
                the two hwdge queues (sync + the now-idle scalar) so no
                slow swdge drain sits at the very end."""
                qsl = slice(qb * QB, (qb + 1) * QB)
                oacc = oaccs.pop(qb)
                nc.vector.tensor_copy(oall[:, qsl], oacc[0:D, :])
                # ACT is idle once the exp stream ends: it takes the l copy
                # and the half-1 cast so both tail chains run in parallel
                # with Vector's (oacc evict + half-0 cast).
                nc.scalar.activation(l_sb[:, qsl], oacc[LCOL:LCOL + 1, :],
                                     mybir.ActivationFunctionType.Copy)
                nc.sync.dma_start(lT[:, qsl], l_sb[:, qsl])
                for half, ofs in ((0, 0), (1, 128)):
                    pj = psX.tile([128, QB], F32, name="pjl", tag="px")
                    ob = obp.tile([128, QB], F16, name="obl", tag="ob")
                    mm(pj[:], wo_sb[:, ofs:ofs + 128], oall[:, qsl],
                       start=True, stop=True)
                    if half == 0:
                        nc.vector.tensor_copy(ob[:], pj[:])
                    else:
                        nc.scalar.activation(
                            ob[:], pj[:], mybir.ActivationFunctionType.Copy)
                    q0 = qb * QB
                    eng = (nc.sync, nc.scalar)
                    eng[half].dma_start(outT[ofs:ofs + 128, q0:q0 + 256],
                                        ob[:, 0:256])
                    eng[1 - half].dma_start(
                        outT[ofs:ofs + 128, q0 + 256:q0 + 512],
                        ob[:, 256:512])

            # PE p-state warmup: dummy matmuls on the first-arrived weights
            # bridge the gap until the x chunk-0 DMA lands (so qk0 doesn't
            # run at the cold 0.65GHz p-state). A few more after qk0 keep
            # the PE busy while the DVE bias-add produces qTt/kTt chunk 0,
            # preserving the p-state ramp into the score stream.
            warm = psX.tile([128, 256], F32, name="warm", tag="px")
            for _ in range(10):
                mm(warm[:], wqk_sb[:, 0:128], wqk_sb[:], start=True, stop=True)
            emit_qk(0, psX, on_act=True)
            for _ in range(4):
                mm(warm[:], wqk_sb[:, 0:128], wqk_sb[:], start=True, stop=True)

            # ---- injected work, placed just ahead of each deadline:
            # kTt chunk c feeds score groups 2c..2c+1 -> qk(c) at group
            # 2c-1; vb chunk c is first read by PV(2c) which drains at
            # group 2c+PVLAG -> emit_v(c) at 2c+2. PV is lagged by a deep
            # PVLAG=10 so qb0 carries no PV work at all -> the PE (which
            # also runs all the injected projections at the not-yet-ramped
            # p-state) can keep the score stream ahead of ACT.
            # NG=11 deadlines: group g covers key tiles 3g..3g+2, so kTt
            # chunk c (tiles 4c..4c+3) is first read at group ceil(4c/3)-ish;
            # vb chunk c is first read by the PV that drains at +PVLAG.
            inject = {
                0: [lambda: emit_bias_dma(2), lambda: emit_qk(1, psAcc)],
                1: [lambda: emit_bias_dma(3), lambda: emit_qk(2, psX)],
                2: [lambda: emit_bias_dma(4), lambda: emit_v(0)],
                3: [lambda: emit_bias_dma(5), lambda: emit_qk(3, psAcc)],
                4: [lambda: emit_qk(4, psAcc)],
                5: [lambda: emit_bias_dma(6), lambda: emit_qk(5, psAcc)],
                6: [lambda: emit_bias_dma(7), lambda: emit_v(1)],
                7: [lambda: emit_qk(6, psX)],
                8: [lambda: nc.gpsimd.dma_start(wo_sb[:], woT[:]),
                    lambda: emit_qk(7, psX)],
                9: [lambda: emit_v(2)],
                10: [lambda: emit_v(3)],
            }
            inject_qb1 = {
                0: [lambda: emit_v(4)],
                2: [lambda: emit_v(5)],
                4: [lambda: emit_v(6)],
                6: [lambda: emit_v(7)],
            }

            # ---- attention stream (PV lags scores by PVLAG groups) ----
            PVLAG = 10
            pv_queue = []
            pending_proj = []

            def emit_pv(qb, gi, p3):
                if gi == 0:
                    oaccs[qb] = psAcc.tile([DV, QB], F32, name="oacc",
                                           tag="oacc")
                oacc = oaccs[qb]
                # fp8 DoubleRow covers tile PAIRS (2x128 contraction at 0.5
                # cycles/row); a 3-tile group is one pair + one plain fp8 mm.
                j, gsz = group_tiles(gi)
                last = j + gsz == NKT
                if gsz >= 2:
                    mm(oacc[:], vb[:, j:j + 2, :], p3[:, 0:2, :],
                       start=(j == 0), stop=(last and gsz == 2),
                       perf_mode=DoubleRow)
                if gsz == 3:
                    mm(oacc[:], vb[:, j + 2:j + 3, :], p3[:, 2:3, :],
                       start=False, stop=last)

            done_qb = {}

            def drain_one_pv():
                qb0_, gi0_, p30_ = pv_queue.pop(0)
                emit_pv(qb0_, gi0_, p30_)
                if gi0_ == NG - 1:
                    done_qb[qb0_] = True

            for qb in range(NQB):
                qsl = slice(qb * QB, (qb + 1) * QB)
                for gi in range(NG):
                    if qb == 0:
                        for thunk in inject.get(gi, ()):
                            thunk()
                    elif qb == 1:
                        for thunk in inject_qb1.get(gi, ()):
                            thunk()
                    j0, gsz = group_tiles(gi)
                    s3 = psB.tile([128, gsz, QB], F32, name="s3", tag="s3")
                    for m in range(gsz):
                        j = j0 + m
                        jl = slice(j * KT, (j + 1) * KT)
                        mm(s3[:, m:m + 1, :], kTt[:, jl], qTt[:, qsl],
                           start=True, stop=True)
                    p3 = ptp.tile([128, gsz, QB], F8E4, name="p3", tag="p3")
                    # flat (depth-1) APs for the ACT engine; the 3D tile
                    # shape only matters for the DoubleRow PV operand view
                    nc.scalar.activation(p3[:].rearrange("p a b -> p (a b)"),
                                         s3[:].rearrange("p a b -> p (a b)"),
                                         Exp)
                    pv_queue.append((qb, gi, p3))
                    # Last three blocks: taper the PV lag gradually (the
                    # extra drains sit mid-block, away from the boundary
                    # where the PE p-state is still recovering) so the
                    # post-stream tail is one PV group instead of PVLAG+1.
                    if qb < NQB - 3:
                        lag = PVLAG
                    elif qb == NQB - 3:
                        lag = PVLAG - min(2, max(0, gi - 8))
                    elif qb == NQB - 2:
                        lag = 8 - min(3, max(0, gi - 4))
                    else:
                        lag = 5 - min(4, max(0, gi - 3))
                    drains = 0
                    while len(pv_queue) > lag and drains < 2:
                        drain_one_pv()
                        drains += 1
                        # previous block done accumulating? emit its oacc
                        # eviction IMMEDIATELY (before a subsequent drain
                        # allocates the next oacc from the single-buffer
                        # pool) and queue the projection steps.
                        if done_qb.pop(qb - 1, None):
                            pending_proj.extend(proj_steps(qb - 1))
                    if pending_proj and (gi % 5 == 3 or
                                         (qb == NQB - 1 and gi % 3 == 2)):
                        pending_proj.pop(0)()
            while pv_queue:
                drain_one_pv()
            for step in pending_proj:
                step()
            proj_last(NQB - 1)


_NC_CACHE = {}


def _get_program():
    if "nc" not in _NC_CACHE:
        _NC_CACHE["nc"] = build_program()
    return _NC_CACHE["nc"]


def make_in_maps(x, codons, syn_bias, wq, bq, wk, bk, wv, bv, wo):
    in_maps = []
    for core in range(8):
        b, h = divmod(core, NH)
        hsl = slice(h * D, (h + 1) * D)
        cod = codons[b]
        onehotT = np.zeros((D, S), np.float16)
        onehotT[cod, np.arange(S)] = 1.0
        # [wqT/8 | wkT] packed as [hi-half ; lo-half] -> [128, 256]
        wqk_full = np.concatenate([wq[hsl, :].T / 8.0, wk[hsl, :].T], axis=1)
        wqk = np.concatenate([wqk_full[0:128], wqk_full[128:256]], axis=1)
        wvp = np.concatenate(
            [wv[hsl, :].T, np.zeros((HID, DV - D), np.float32)], axis=1)
        wv2 = np.concatenate([wvp[0:128], wvp[128:256]], axis=1)
        bias2 = np.zeros((128, 2), np.float32)
        bias2[:, 0] = np.concatenate([bq[hsl] / 8.0, bk[hsl]])
        in_maps.append({
            "xT": x[b].T.astype(np.float16),
            "wqk": wqk.astype(np.float16),
            "wv2": wv2.astype(np.float16),
            "bias2": bias2,
            "bsynT": np.ascontiguousarray(syn_bias.T[:, cod]).astype(np.float16),
            "onehotT": onehotT,
            "woT": wo[:, hsl].T.astype(np.float16),
        })
    return in_maps


def kernel_run(inputs, trace=False):
    x = np.asarray(inputs["x"], np.float32)
    codons = np.asarray(inputs["codons"]).astype(np.int64)
    syn_bias = np.asarray(inputs["syn_bias"], np.float32)
    wq = np.asarray(inputs["wq"], np.float32)
    bq = np.asarray(inputs["bq"], np.float32)
    wk = np.asarray(inputs["wk"], np.float32)
    bk = np.asarray(inputs["bk"], np.float32)
    wv = np.asarray(inputs["wv"], np.float32)
    bv = np.asarray(inputs["bv"], np.float32)
    wo = np.asarray(inputs["wo"], np.float32)
    bo = np.asarray(inputs["bo"], np.float32)

    nc = _get_program()
    in_maps = make_in_maps(x, codons, syn_bias, wq, bq, wk, bk, wv, bv, wo)
    res = run_bass_kernel_spmd(nc, in_maps, core_ids=list(range(8)), trace=trace)

    # v' is computed WITHOUT bv on-device; after normalization the bias
    # contributes exactly wo @ bv per sequence position, so fold it into bo.
    bo_eff = bo + wo @ bv
    out = np.empty((B, S, HID), np.float32)
    for b in range(B):
        acc = None
        for h in range(NH):
            r = res.results[NH * b + h]
            part = r["outT"].astype(np.float32) / r["lT"]   # normalize per head
            acc = part if acc is None else acc + part
        out[b] = acc.T + bo_eff
    return out, res


def kernel(**inputs):
    out, _ = kernel_run(inputs, trace=False)
    return out

